# revision 1
# baseline (speedup 1.0000x reference)
"""Trainium2 Bass kernel for nn_Correct_PrototypeManager (segment_reduce).

Reference computation:
    pred_lbl = argmax(preds, axis=1)                      # [B, H, W]
    feats_up = bilinear_resize(feats, H, W)               # [B, C, H, W]
    joint[b,k,h,w] = (masks==k) & (pred_lbl==k)
    counts[b,k] = sum_hw joint ; sums[b,k,c] = sum_hw feats_up * joint
    proto = mean_b( sums / (counts + eps) )               # [K, C]

Key algebraic transform used on-device: bilinear upsample is linear,
feats_up = (Uh (x) Uw) @ feats, so
    sums[k,c] = <joint_k, U feats_c> = <U^T joint_k, feats_c>
We downsample the one-hot joint map (256^2 -> 64^2 pixels) with the
adjoint of the upsample and contract over only 4096 coarse pixels.
counts are preserved exactly because rows of U sum to 1.

Sharding: data-parallel over batch B=8, one image per NeuronCore; the
[C+1, K] per-image partial (sums^T stacked with counts) is gathered on
host, divided and batch-meaned there (tiny).
"""

import numpy as np

B = 8
C = 256
K = 21
HC = WC = 64
HF = WF = 256
EPS = 1e-6
N_CORES = 8
PIX = HC * WC  # 4096

_PROGRAM_CACHE: dict = {}


def _upsample_matrix(n_in: int, n_out: int) -> np.ndarray:
    """U [n_out, n_in] with resize(x, 'bilinear', half-pixel) == U @ x."""
    U = np.zeros((n_out, n_in), dtype=np.float64)
    scale = n_in / n_out
    for i in range(n_out):
        src = (i + 0.5) * scale - 0.5
        f = int(np.floor(src))
        w = src - f
        lo = min(max(f, 0), n_in - 1)
        hi = min(max(f + 1, 0), n_in - 1)
        U[i, lo] += 1.0 - w
        U[i, hi] += w
    return U.astype(np.float32)


def _build_program(stage: int = 99):
    import concourse.bass as bass
    import concourse.bacc as bacc
    import concourse.tile as tile
    from concourse import mybir
    from contextlib import ExitStack

    f32 = mybir.dt.float32
    bf16 = mybir.dt.bfloat16

    nc = bacc.Bacc("TRN2", target_bir_lowering=False, debug=False,
                   num_devices=N_CORES)

    preds_d = nc.dram_tensor("preds", [K, HF, WF], f32, kind="ExternalInput")
    feats_d = nc.dram_tensor("feats", [C, PIX], f32, kind="ExternalInput")
    # mask packed with the iota row per hf-half: one DMA per half so the
    # one-hot TensorTensor needs a single sync wait (TT has 1 wait slot).
    mask_d = nc.dram_tensor("mask", [2, 128, WF + K], bf16,
                            kind="ExternalInput")
    u16_d = nc.dram_tensor("u16", [HF, HC], bf16, kind="ExternalInput")
    u32_d = nc.dram_tensor("u32", [HF, HC], f32, kind="ExternalInput")
    ident_d = nc.dram_tensor("ident", [128, 128], f32, kind="ExternalInput")
    out_d = nc.dram_tensor("out", [C + 1, K], f32, kind="ExternalOutput")

    KW = K * WF  # 5376 free elems per half: (k, wf) k-major
    K2 = K + 1   # pad class dim to even: fp32 matmuls need even free dims
    HK = HC * K2  # 1408: (hc, k2) hc-major

    with tile.TileContext(nc) as tc, ExitStack() as ctx:
        const_pool = ctx.enter_context(tc.tile_pool(name="const", bufs=1))
        joint_pool = ctx.enter_context(tc.tile_pool(name="joint", bufs=2))
        ft_pool = ctx.enter_context(tc.tile_pool(name="ft", bufs=1))
        res_pool = ctx.enter_context(tc.tile_pool(name="res", bufs=1))
        ps_pool = ctx.enter_context(
            tc.tile_pool(name="ps", bufs=4, space="PSUM"))
        psf_pool = ctx.enter_context(
            tc.tile_pool(name="psf", bufs=1, space="PSUM"))

        # --- constants ---
        u16_t = []
        u32_t = []
        for h in range(2):
            t16 = const_pool.tile([128, HC], bf16, tag=f"u16_{h}")
            nc.sync.dma_start(t16[:], u16_d.ap()[h * 128:(h + 1) * 128, :])
            u16_t.append(t16)
            t32 = const_pool.tile([128, HC], f32, tag=f"u32_{h}")
            nc.sync.dma_start(t32[:], u32_d.ap()[h * 128:(h + 1) * 128, :])
            u32_t.append(t32)
        ident_t = const_pool.tile([128, 128], f32, tag="ident")
        nc.sync.dma_start(ident_t[:], ident_d.ap()[:, :])
        ones_t = const_pool.tile([64, 2], f32, tag="ones")
        nc.vector.memset(ones_t[:], 1.0)

        # ft_big[pix_part, (px 32, c2 2, 128)] : feats^T, pix on partitions
        ft_big = ft_pool.tile([128, 32 * 256], f32, tag="ftbig")

        joint_t = []

        with tc.tile_pool(name="trans", bufs=2) as tr_pool:
            # --- load + build joint (per hf half) ---
            for h in range(2):
                hs = h * 128
                preds_t = tr_pool.tile([128, KW], f32, tag="preds")
                nc.sync.dma_start(
                    preds_t[:],
                    preds_d.ap()[:, hs:hs + 128, :].transpose([1, 0, 2]))
                mask_t = tr_pool.tile([128, WF + K], bf16, tag="mask")
                nc.sync.dma_start(mask_t[:], mask_d.ap()[h, :, :])

                preds3 = preds_t[:].rearrange("p (k w) -> p k w", k=K)
                maxv_t = tr_pool.tile([128, WF], f32, tag="maxv")
                nc.vector.tensor_reduce(
                    maxv_t[:], preds3.transpose([0, 2, 1]),
                    axis=mybir.AxisListType.X, op=mybir.AluOpType.max)

                eq_t = tr_pool.tile([128, KW], bf16, tag="eq")
                nc.vector.tensor_tensor(
                    eq_t[:].rearrange("p (k w) -> p k w", k=K),
                    preds3,
                    maxv_t[:].unsqueeze(1).to_broadcast([128, K, WF]),
                    op=mybir.AluOpType.is_equal)

                oh_t = tr_pool.tile([128, KW], bf16, tag="oh")
                nc.vector.tensor_tensor(
                    oh_t[:].rearrange("p (k w) -> p k w", k=K),
                    mask_t[:, :WF].unsqueeze(1).to_broadcast([128, K, WF]),
                    mask_t[:, WF:WF + K].unsqueeze(2).to_broadcast(
                        [128, K, WF]),
                    op=mybir.AluOpType.is_equal)

                jt = joint_pool.tile([128, KW], bf16, tag=f"joint{h}")
                nc.vector.tensor_mul(jt[:], eq_t[:], oh_t[:])
                joint_t.append(jt)

            if stage <= 1:  # debug: dump joint slice (gpsimd casts bf16->f32)
                nc.gpsimd.dma_start(out_d.ap()[0:128, :],
                                    joint_t[0][:, 0:K])
                nc.gpsimd.dma_start(out_d.ap()[128:256, :],
                                    joint_t[1][:, 0:K])
                nc.gpsimd.dma_start(out_d.ap()[C:C + 1, :],
                                    joint_t[0][0:1, K:2 * K])

            # --- feats load + transpose (PE) into ft_big ---
            for c2 in range(2 if stage >= 2 else 0):
                feats_t = tr_pool.tile([128, PIX], f32, tag="feats")
                nc.sync.dma_start(
                    feats_t[:], feats_d.ap()[c2 * 128:(c2 + 1) * 128, :])
                for pg in range(8):  # groups of 4 pix-chunks
                    ps = ps_pool.tile([128, 512], f32, tag="ps")
                    for j in range(4):
                        px = pg * 4 + j
                        nc.tensor.transpose(
                            ps[:, j * 128:(j + 1) * 128],
                            feats_t[:, px * 128:(px + 1) * 128],
                            ident_t[:])
                    # dest: ft_big[:, px*256 + c2*128 + 0:128] for 4 px
                    dst = ft_big[:].rearrange(
                        "p (x c n) -> p x c n", x=32, c=2)[
                            :, pg * 4:(pg + 1) * 4, c2, :]
                    nc.scalar.copy(dst, ps[:].rearrange(
                        "p (j n) -> p j n", j=4))

        # ----- stage 1: contract hf.  A[hc, (k, wf)] = Uh^T @ joint -----
        with tc.tile_pool(name="stage", bufs=1) as st_pool:
            a_t = st_pool.tile([64, KW], f32, tag="a")
            for fc in range(0, KW if stage >= 3 else 0, 512):
                w = min(512, KW - fc)
                ps = ps_pool.tile([64, 512], f32, tag="ps")
                nc.tensor.matmul(ps[:, :w], u16_t[0][:, :],
                                 joint_t[0][:, fc:fc + w],
                                 start=True, stop=False)
                nc.tensor.matmul(ps[:, :w], u16_t[1][:, :],
                                 joint_t[1][:, fc:fc + w],
                                 start=False, stop=True)
                nc.scalar.copy(a_t[:, fc:fc + w], ps[:, :w])

            # ----- stage 1.5: transpose A per class -> AT[wf, (wh,(hc,k2))]
            at_big = st_pool.tile([128, 2 * HK], f32, tag="at")
            if stage >= 4:
                # zero the k=21 pad column so stage 2 produces clean zeros
                nc.vector.memset(
                    at_big[:].rearrange(
                        "p (w h k) -> p w h k", w=2, h=HC)[:, :, :, K], 0.0)
            for k in range(K if stage >= 4 else 0):
                ps = ps_pool.tile([128, 128], f32, tag="ps")
                for wh in range(2):
                    nc.tensor.transpose(
                        ps[:, wh * 64:(wh + 1) * 64],
                        a_t[:, k * WF + wh * 128: k * WF + wh * 128 + 128],
                        ident_t[:64, :64])
                dst = at_big[:].rearrange(
                    "p (w h k) -> p w h k", w=2, h=HC)[:, :, :, k]
                nc.scalar.copy(dst, ps[:].rearrange("p (w h) -> p w h", w=2))

            # ----- stage 2: contract wf.  B[wc, (hc, k)] = Uw^T @ AT -----
            # B lives twice (partitions 0-63 and 64-127) so the final
            # matmuls can match the base partition of the FT slice.
            b_t = st_pool.tile([128, HK], f32, tag="b")
            for fc in range(0, HK if stage >= 4 else 0, 512):
                w = min(512, HK - fc)
                ps = ps_pool.tile([64, 512], f32, tag="ps")
                nc.tensor.matmul(ps[:, :w], u32_t[0][:, :],
                                 at_big[:, fc:fc + w],
                                 start=True, stop=False)
                nc.tensor.matmul(ps[:, :w], u32_t[1][:, :],
                                 at_big[:, HK + fc:HK + fc + w],
                                 start=False, stop=True)
                nc.scalar.copy(b_t[0:64, fc:fc + w], ps[:, :w])
            if stage >= 4:
                # partitions 64-127 hold B shifted by one hc, so one 128-pixel
                # chunk (two hc rows) is a single full-partition matmul slice
                nc.sync.dma_start(b_t[64:128, 0:HK - K2], b_t[0:64, K2:HK])

            # ----- final: sums^T[c,k] = sum_hc FT_hc^T @ B_hc, counts -----
            ftv = ft_big[:].rearrange("p (x n) -> p x n", x=32)
            for c2 in range(2 if stage >= 5 else 0):
                psum_c = psf_pool.tile([128, K2], f32, tag=f"fin{c2}")
                for ch in range(32):
                    nc.tensor.matmul(
                        psum_c[:, :],
                        ftv[:, ch, c2 * 128:(c2 + 1) * 128],
                        b_t[:, 2 * ch * K2:2 * ch * K2 + K2],
                        start=(ch == 0), stop=(ch == 31))
                outc = res_pool.tile([128, K], f32, tag=f"outc{c2}")
                nc.scalar.copy(outc[:], psum_c[:, 0:K])
                nc.sync.dma_start(
                    out_d.ap()[c2 * 128:(c2 + 1) * 128, :], outc[:])

            if stage >= 5:
                cntp = res_pool.tile([64, K2], f32, tag="cntp")
                nc.vector.tensor_reduce(
                    cntp[:],
                    b_t[0:64, :].rearrange("p (h k) -> p k h", k=K2),
                    axis=mybir.AxisListType.X, op=mybir.AluOpType.add)
                psum_n = psf_pool.tile([2, K2], f32, tag="fincnt")
                nc.tensor.matmul(psum_n[:], ones_t[:], cntp[:],
                                 start=True, stop=True)
                cnt_sb = res_pool.tile([1, K], f32, tag="cnt")
                nc.scalar.copy(cnt_sb[:], psum_n[0:1, 0:K])
                nc.sync.dma_start(out_d.ap()[C:C + 1, :], cnt_sb[:])

    nc.compile()
    return nc


def _get_program():
    if "nc" not in _PROGRAM_CACHE:
        _PROGRAM_CACHE["nc"] = _build_program()
    return _PROGRAM_CACHE["nc"]


def _host_inputs(feats, preds, masks):
    import ml_dtypes

    U = _upsample_matrix(HC, HF)
    u16 = U.astype(ml_dtypes.bfloat16)
    ident = np.eye(128, dtype=np.float32)

    feats = np.asarray(feats, dtype=np.float32)
    preds = np.asarray(preds, dtype=np.float32)
    masks_f = np.asarray(masks).astype(np.float32)
    iota_row = np.arange(K, dtype=np.float32)
    # [B, 2, 128, WF+K]: mask halves with the iota row appended
    mio = np.empty((B, 2, 128, WF + K), dtype=np.float32)
    mio[..., :WF] = masks_f.reshape(B, 2, 128, WF)
    mio[..., WF:] = iota_row
    mio_bf = mio.astype(ml_dtypes.bfloat16)

    in_maps = []
    for b in range(B):
        in_maps.append({
            "preds": np.ascontiguousarray(preds[b]),
            "feats": np.ascontiguousarray(feats[b].reshape(C, PIX)),
            "mask": np.ascontiguousarray(mio_bf[b]),
            "u16": u16,
            "u32": U,
            "ident": ident,
        })
    return in_maps


def kernel(feats, preds, masks, _results_hook=None):
    from concourse.bass_utils import run_bass_kernel_spmd

    nc = _get_program()
    in_maps = _host_inputs(feats, preds, masks)
    res = run_bass_kernel_spmd(nc, in_maps, list(range(N_CORES)))
    if _results_hook is not None:
        _results_hook(res)

    protos = []
    for b in range(B):
        out = res.results[b]["out"]  # [C+1, K] f32
        sums_t = out[:C, :]          # [C, K]
        counts = out[C, :]           # [K]
        protos.append((sums_t / (counts + EPS)[None, :]).T)  # [K, C]
    return np.mean(np.stack(protos), axis=0).astype(np.float32)



# revision 8
# speedup vs baseline: 1.1938x; 1.1938x over previous
"""Trainium2 Bass kernel for nn_Correct_PrototypeManager (segment_reduce).

Reference computation:
    pred_lbl = argmax(preds, axis=1)                      # [B, H, W]
    feats_up = bilinear_resize(feats, H, W)               # [B, C, H, W]
    joint[b,k,h,w] = (masks==k) & (pred_lbl==k)
    counts[b,k] = sum_hw joint ; sums[b,k,c] = sum_hw feats_up * joint
    proto = mean_b( sums / (counts + eps) )               # [K, C]

Algebraic transform: bilinear upsample is linear (feats_up = (Uh (x) Uw)
@ feats), so sums[k,c] = <Uh^T joint_k Uw, feats_c> — we downsample the
one-hot joint map (256^2 -> 64^2) with the adjoint of the upsample and
contract over 4096 coarse pixels. counts are preserved exactly (rows of
U sum to 1); counts are obtained from the same final matmul via a ones
column appended to the feats operand.

Numerics: the whole downsample pipeline runs in fp16 EXACTLY — joint is
0/1, bilinear adjoint weights are multiples of 1/8, so A (<=4, units of
1/8) and B (<=16, units of 1/64) fit fp16's 11-bit mantissa. The argmax
compare stays fp32 (fp16 would create false ties). Only feats are
rounded to fp16 (~2.4e-4 relative).

Sharding: data-parallel over batch B=8, one image per NeuronCore; the
[22, 258] per-image partial (sums[k,c] + counts col) is gathered on
host, divided and batch-meaned there (tiny).
"""

import numpy as np

B = 8
C = 256
K = 21
K2 = K + 1          # pad class dim (fp32-even + contiguous b2 blocks)
HC = WC = 64
HF = WF = 256
EPS = 1e-6
N_CORES = 8
PIX = HC * WC       # 4096
KW = K * WF         # 5376 joint free elems per half
HK = HC * K2        # 1408 b2 free elems
WFB = 128           # wf pipeline block width
NB = WF // WFB      # wf blocks per half
FTW = C + 2         # ft chunk width: 256 feats + ones col + pad = 258

_PROGRAM_CACHE: dict = {}


def _upsample_matrix(n_in: int, n_out: int) -> np.ndarray:
    """U [n_out, n_in] with resize(x, 'bilinear', half-pixel) == U @ x."""
    U = np.zeros((n_out, n_in), dtype=np.float64)
    scale = n_in / n_out
    for i in range(n_out):
        src = (i + 0.5) * scale - 0.5
        f = int(np.floor(src))
        w = src - f
        lo = min(max(f, 0), n_in - 1)
        hi = min(max(f + 1, 0), n_in - 1)
        U[i, lo] += 1.0 - w
        U[i, hi] += w
    return U.astype(np.float32)


def _build_program(stage: int = 99):
    import concourse.bass as bass
    import concourse.bacc as bacc
    import concourse.tile as tile
    from concourse import mybir
    from contextlib import ExitStack

    f32 = mybir.dt.float32
    f16 = mybir.dt.float16

    nc = bacc.Bacc("TRN2", target_bir_lowering=False, debug=False,
                   num_devices=N_CORES)

    preds_d = nc.dram_tensor("preds", [K, HF, WF], f32, kind="ExternalInput")
    mask_d = nc.dram_tensor("mask", [2, 128, WF], f16, kind="ExternalInput")
    iota_d = nc.dram_tensor("iota", [128, K * WFB], f16, kind="ExternalInput")
    ft_d = nc.dram_tensor("ft", [PIX // 128, 128, FTW], f16,
                          kind="ExternalInput")
    u16_d = nc.dram_tensor("u16", [HF, HC], f16, kind="ExternalInput")
    u32_d = nc.dram_tensor("u32", [HF, HC], f16, kind="ExternalInput")
    ident_d = nc.dram_tensor("ident", [128, 128], f32, kind="ExternalInput")
    out_d = nc.dram_tensor("out", [K2, FTW], f32, kind="ExternalOutput")

    with tile.TileContext(nc) as tc, ExitStack() as ctx:
        const_pool = ctx.enter_context(tc.tile_pool(name="const", bufs=1))
        joint_pool = ctx.enter_context(tc.tile_pool(name="joint", bufs=1))
        ft_pool = ctx.enter_context(tc.tile_pool(name="ft", bufs=1))
        res_pool = ctx.enter_context(tc.tile_pool(name="res", bufs=1))
        ps_pool = ctx.enter_context(
            tc.tile_pool(name="ps", bufs=4, space="PSUM"))
        psf_pool = ctx.enter_context(
            tc.tile_pool(name="psf", bufs=1, space="PSUM"))

        # --- constants / independent DMAs (lead-in) ---
        u16_t = []
        u32_t = []
        for h in range(2):
            t16 = const_pool.tile([128, HC], f16, tag=f"u16_{h}")
            nc.sync.dma_start(t16[:], u16_d.ap()[h * 128:(h + 1) * 128, :])
            u16_t.append(t16)
            t32 = const_pool.tile([128, HC], f16, tag=f"u32_{h}")
            nc.sync.dma_start(t32[:], u32_d.ap()[h * 128:(h + 1) * 128, :])
            u32_t.append(t32)
        ident_t = const_pool.tile([128, 128], f32, tag="ident")
        nc.sync.dma_start(ident_t[:], ident_d.ap()[:, :])
        iota_t = const_pool.tile([128, K * WFB], f16, tag="iota")
        nc.sync.dma_start(iota_t[:], iota_d.ap()[:, :])
        mask_t = []
        for h in range(2):
            mt = const_pool.tile([128, WF], f16, tag=f"mask{h}")
            nc.sync.dma_start(mt[:], mask_d.ap()[h, :, :])
            mask_t.append(mt)

        # --- one-hot of mask (independent of preds; runs in DMA shadow) ---
        # oh[p, k, w] = (mask[p, w] == k), fp16 exact, DVE 2x mode
        oh_t = []
        for h in range(2):
            oh = joint_pool.tile([128, KW], f16, tag=f"oh{h}")
            oh_t.append(oh)
            for b in range(NB):
                ws = b * WFB
                nc.vector.tensor_tensor(
                    oh[:].rearrange("p (k w) -> p k w", k=K)[:, :, ws:ws + WFB],
                    mask_t[h][:, ws:ws + WFB].unsqueeze(1).to_broadcast(
                        [128, K, WFB]),
                    iota_t[:].rearrange("p (k w) -> p k w", k=K),
                    op=mybir.AluOpType.is_equal)

        # --- preds load + argmax-eq + joint, per (half, wf-block) ---
        joint_t = []
        with tc.tile_pool(name="trans", bufs=2) as tr_pool, \
                tc.tile_pool(name="mx", bufs=4) as mx_pool:
            preds_t = []
            for h in range(2):
                hs = h * 128
                pt = tr_pool.tile([128, KW], f32, tag=f"preds{h}")
                preds_t.append(pt)
                for b in range(NB):
                    ws = b * WFB
                    nc.sync.dma_start(
                        pt[:].rearrange("p (k w) -> p k w", k=K)[
                            :, :, ws:ws + WFB],
                        preds_d.ap()[:, hs:hs + 128, ws:ws + WFB].transpose(
                            [1, 0, 2]))

            # feats^T (pix on partitions, hc-major) with ones+pad cols, fp16.
            # Issued after the preds DMAs: only the final matmuls need it.
            ft_big = ft_pool.tile([128, (PIX // 128) * FTW], f16, tag="ftbig")
            nc.sync.dma_start(
                ft_big[:].rearrange("p (x n) -> p x n", x=PIX // 128),
                ft_d.ap().transpose([1, 0, 2]))

            for h in range(2):
                jt = joint_pool.tile([128, KW], f16, tag=f"joint{h}")
                joint_t.append(jt)
                for b in range(NB):
                    ws = b * WFB
                    p3 = preds_t[h][:].rearrange(
                        "p (k w) -> p k w", k=K)[:, :, ws:ws + WFB]
                    # max over k: split tree DVE (k 0..9) / Pool (k 10..20)
                    t5 = mx_pool.tile([128, 5 * WFB], f32, tag="t5")
                    t5v = t5[:].rearrange("p (k w) -> p k w", k=5)
                    nc.vector.tensor_tensor(
                        t5v, p3[:, 0:5, :], p3[:, 5:10, :],
                        op=mybir.AluOpType.max)
                    nc.vector.tensor_tensor(
                        t5v[:, 0:2, :], t5v[:, 0:2, :], t5v[:, 2:4, :],
                        op=mybir.AluOpType.max)
                    nc.vector.tensor_tensor(
                        t5v[:, 0:1, :], t5v[:, 0:1, :], t5v[:, 1:2, :],
                        op=mybir.AluOpType.max)
                    nc.vector.tensor_tensor(
                        t5v[:, 0:1, :], t5v[:, 0:1, :], t5v[:, 4:5, :],
                        op=mybir.AluOpType.max)
                    g5 = mx_pool.tile([128, 5 * WFB], f32, tag="g5")
                    g5v = g5[:].rearrange("p (k w) -> p k w", k=5)
                    nc.vector.tensor_tensor(
                        g5v, p3[:, 10:15, :], p3[:, 15:20, :],
                        op=mybir.AluOpType.max)
                    nc.vector.tensor_tensor(
                        g5v[:, 0:2, :], g5v[:, 0:2, :], g5v[:, 2:4, :],
                        op=mybir.AluOpType.max)
                    nc.vector.tensor_tensor(
                        g5v[:, 0:1, :], g5v[:, 0:1, :], g5v[:, 1:2, :],
                        op=mybir.AluOpType.max)
                    nc.vector.tensor_tensor(
                        g5v[:, 0:1, :], g5v[:, 0:1, :], g5v[:, 4:5, :],
                        op=mybir.AluOpType.max)
                    nc.vector.tensor_tensor(
                        g5v[:, 0:1, :], g5v[:, 0:1, :], p3[:, 20:21, :],
                        op=mybir.AluOpType.max)
                    mxv = mx_pool.tile([128, WFB], f32, tag="mxv")
                    nc.vector.tensor_tensor(
                        mxv[:].unsqueeze(1), t5v[:, 0:1, :], g5v[:, 0:1, :],
                        op=mybir.AluOpType.max)

                    # eq = (preds == maxv): fp32 compare, fp16 0/1 out (DVE;
                    # the Pool engine has no is_equal)
                    jv = jt[:].rearrange("p (k w) -> p k w", k=K)[
                        :, :, ws:ws + WFB]
                    nc.vector.tensor_tensor(
                        jv, p3,
                        mxv[:].unsqueeze(1).to_broadcast([128, K, WFB]),
                        op=mybir.AluOpType.is_equal)
                    # joint = eq * oh, fp16, on Pool (keeps DVE free); the
                    # very last block is k-split so its tail is short
                    ov = oh_t[h][:].rearrange("p (k w) -> p k w", k=K)[
                        :, :, ws:ws + WFB]
                    if h == 1 and b == NB - 1:
                        KS = 14
                        nc.gpsimd.tensor_tensor(
                            jv[:, 0:KS, :], jv[:, 0:KS, :], ov[:, 0:KS, :],
                            op=mybir.AluOpType.mult)
                        nc.vector.tensor_tensor(
                            jv[:, KS:K, :], jv[:, KS:K, :], ov[:, KS:K, :],
                            op=mybir.AluOpType.mult)
                    else:
                        nc.gpsimd.tensor_tensor(
                            jv, jv, ov, op=mybir.AluOpType.mult)

        if stage <= 1:  # debug: dump joint slice for classes 0..K2-1
            dbg = res_pool.tile([128, K2], f32, tag="dbg")
            nc.scalar.copy(dbg[:], joint_t[0][:, 0:K2])
            nc.sync.dma_start(out_d.ap()[:, 0:128].transpose([1, 0]), dbg[:])

        # ----- stage 1: contract hf.  A[hc, (k, wf)] = Uh^T @ joint -----
        with tc.tile_pool(name="stg", bufs=1) as st_pool:
            a_t = st_pool.tile([64, KW], f32, tag="a")
            for fc in range(0, KW if stage >= 2 else 0, 512):
                w = min(512, KW - fc)
                ps = ps_pool.tile([64, 512], f32, tag="ps")
                nc.tensor.matmul(ps[:, :w], u16_t[0][:, :],
                                 joint_t[0][:, fc:fc + w],
                                 start=True, stop=False)
                nc.tensor.matmul(ps[:, :w], u16_t[1][:, :],
                                 joint_t[1][:, fc:fc + w],
                                 start=False, stop=True)
                nc.scalar.copy(a_t[:, fc:fc + w], ps[:, :w])

            # ----- stage 1.5: transpose A per class -> AT[wf, (wh, hc, k2)]
            at_big = st_pool.tile([128, 2 * HK], f16, tag="at")
            if stage >= 3:
                # zero the k=21 pad column so stage 2 reads clean zeros
                nc.vector.memset(
                    at_big[:].rearrange(
                        "p (w h k) -> p w h k", w=2, h=HC)[:, :, :, K], 0.0)
            for k in range(K if stage >= 3 else 0):
                ps = ps_pool.tile([128, 128], f32, tag="ps")
                for wh in range(2):
                    nc.tensor.transpose(
                        ps[:, wh * 64:(wh + 1) * 64],
                        a_t[:, k * WF + wh * 128: k * WF + wh * 128 + 128],
                        ident_t[:64, :64])
                dst = at_big[:].rearrange(
                    "p (w h k) -> p w h k", w=2, h=HC)[:, :, :, k]
                nc.scalar.copy(dst, ps[:].rearrange("p (w h) -> p w h", w=2))

            # ----- stage 2: contract wf.  B[wc, (hc, k2)] = Uw^T @ AT -----
            b2 = st_pool.tile([128, HK], f16, tag="b2")
            for fc in range(0, HK if stage >= 3 else 0, 512):
                w = min(512, HK - fc)
                ps = ps_pool.tile([64, 512], f32, tag="ps")
                nc.tensor.matmul(ps[:, :w], u32_t[0][:, :],
                                 at_big[:, fc:fc + w],
                                 start=True, stop=False)
                nc.tensor.matmul(ps[:, :w], u32_t[1][:, :],
                                 at_big[:, HK + fc:HK + fc + w],
                                 start=False, stop=True)
                nc.scalar.copy(b2[0:64, fc:fc + w], ps[:, :w])
            if stage >= 3:
                # partitions 64-127 hold B shifted by one hc so a 128-pixel
                # chunk (2 hc rows x 64 wc) is one full-partition stat slice
                nc.sync.dma_start(b2[64:128, 0:HK - K2], b2[0:64, K2:HK])

            # ----- final: out[k, c] = sum_ch b2_ch^T @ ft_ch, + counts ----
            ftv = ft_big[:].rearrange("p (x n) -> p x n", x=PIX // 128)
            if stage >= 4:
                psum_o = psf_pool.tile([K2, FTW], f32, tag="fin")
                for ch in range(PIX // 128):
                    nc.tensor.matmul(
                        psum_o[:, :],
                        b2[:, 2 * ch * K2: 2 * ch * K2 + K2],
                        ftv[:, ch, :],
                        start=(ch == 0), stop=(ch == PIX // 128 - 1))
                outc = res_pool.tile([K2, FTW], f32, tag="outc")
                nc.scalar.copy(outc[:], psum_o[:])
                nc.sync.dma_start(out_d.ap()[:, :], outc[:])

    nc.compile()
    return nc


def _get_program():
    if "nc" not in _PROGRAM_CACHE:
        _PROGRAM_CACHE["nc"] = _build_program()
    return _PROGRAM_CACHE["nc"]


def _host_inputs(feats, preds, masks):
    U = _upsample_matrix(HC, HF)
    u16 = U.astype(np.float16)
    ident = np.eye(128, dtype=np.float32)
    iota = np.broadcast_to(
        np.arange(K, dtype=np.float16)[None, :, None], (128, K, WFB)
    ).reshape(128, K * WFB).copy()

    feats = np.asarray(feats, dtype=np.float32)
    preds = np.asarray(preds, dtype=np.float32)
    masks_f = np.asarray(masks).astype(np.float16).reshape(B, 2, 128, WF)

    # feats^T [pix, c] fp16 with ones + zero-pad cols -> [32, 128, 258]
    ftp = np.empty((B, PIX, FTW), dtype=np.float16)
    ftp[:, :, :C] = feats.reshape(B, C, PIX).transpose(0, 2, 1)
    ftp[:, :, C] = 1.0
    ftp[:, :, C + 1] = 0.0

    in_maps = []
    for b in range(B):
        in_maps.append({
            "preds": np.ascontiguousarray(preds[b]),
            "mask": np.ascontiguousarray(masks_f[b]),
            "iota": iota,
            "ft": np.ascontiguousarray(ftp[b].reshape(PIX // 128, 128, FTW)),
            "u16": u16,
            "u32": u16,
            "ident": ident,
        })
    return in_maps


def kernel(feats, preds, masks, _results_hook=None):
    from concourse.bass_utils import run_bass_kernel_spmd

    nc = _get_program()
    in_maps = _host_inputs(feats, preds, masks)
    res = run_bass_kernel_spmd(nc, in_maps, list(range(N_CORES)))
    if _results_hook is not None:
        _results_hook(res)

    protos = []
    for b in range(B):
        out = res.results[b]["out"]   # [K2, FTW] f32
        sums = out[:K, :C]            # [K, C]
        counts = out[:K, C]           # [K]
        protos.append(sums / (counts + EPS)[:, None])  # [K, C]
    return np.mean(np.stack(protos), axis=0).astype(np.float32)


# revision 11
# speedup vs baseline: 1.4146x; 1.1850x over previous
"""Trainium2 Bass kernel for nn_Correct_PrototypeManager (segment_reduce).

Reference computation:
    pred_lbl = argmax(preds, axis=1)                      # [B, H, W]
    feats_up = bilinear_resize(feats, H, W)               # [B, C, H, W]
    joint[b,k,h,w] = (masks==k) & (pred_lbl==k)
    counts[b,k] = sum_hw joint ; sums[b,k,c] = sum_hw feats_up * joint
    proto = mean_b( sums / (counts + eps) )               # [K, C]

Algebraic transform: bilinear upsample is linear (feats_up = (Uh (x) Uw)
@ feats), so sums[k,c] = <Uh^T joint_k Uw, feats_c> — we downsample the
one-hot joint map (256^2 -> 64^2) with the adjoint of the upsample and
contract over 4096 coarse pixels. counts are preserved exactly (rows of
U sum to 1); counts are obtained from the same final matmul via a ones
column appended to the feats operand.

Numerics: the whole downsample pipeline runs in fp16 EXACTLY — joint is
0/1, bilinear adjoint weights are multiples of 1/8, so A (<=4, units of
1/8) and B (<=16, units of 1/64) fit fp16's 11-bit mantissa. The argmax
compare stays fp32 (fp16 would create false ties). Only feats are
rounded to fp16 (~2.4e-4 relative).

Sharding: data-parallel over batch B=8, one image per NeuronCore; the
[22, 258] per-image partial (sums[k,c] + counts col) is gathered on
host, divided and batch-meaned there (tiny).
"""

import numpy as np

B = 8
C = 256
K = 21
K2 = K + 1          # pad class dim (fp32-even + contiguous b2 blocks)
HC = WC = 64
HF = WF = 256
EPS = 1e-6
N_CORES = 8
PIX = HC * WC       # 4096
KW = K * WF         # 5376 joint free elems per half
HK = HC * K2        # 1408 b2 free elems
WFB = 128           # wf pipeline block width
NB = WF // WFB      # wf blocks per half
FTW = C + 2         # ft chunk width: 256 feats + ones col + pad = 258

_PROGRAM_CACHE: dict = {}


def _upsample_matrix(n_in: int, n_out: int) -> np.ndarray:
    """U [n_out, n_in] with resize(x, 'bilinear', half-pixel) == U @ x."""
    U = np.zeros((n_out, n_in), dtype=np.float64)
    scale = n_in / n_out
    for i in range(n_out):
        src = (i + 0.5) * scale - 0.5
        f = int(np.floor(src))
        w = src - f
        lo = min(max(f, 0), n_in - 1)
        hi = min(max(f + 1, 0), n_in - 1)
        U[i, lo] += 1.0 - w
        U[i, hi] += w
    return U.astype(np.float32)


def _build_program(stage: int = 99):
    import concourse.bass as bass
    import concourse.bacc as bacc
    import concourse.tile as tile
    from concourse import mybir
    from contextlib import ExitStack

    f32 = mybir.dt.float32
    f16 = mybir.dt.float16

    nc = bacc.Bacc("TRN2", target_bir_lowering=False, debug=False,
                   num_devices=N_CORES)

    preds_d = nc.dram_tensor("preds", [K, HF, WF], f32, kind="ExternalInput")
    mask_d = nc.dram_tensor("mask", [2, 128, WF], f16, kind="ExternalInput")
    iota_d = nc.dram_tensor("iota", [128, K * WFB], f16, kind="ExternalInput")
    ft_d = nc.dram_tensor("ft", [PIX // 128, 128, FTW], f16,
                          kind="ExternalInput")
    u16_d = nc.dram_tensor("u16", [HF, HC], f16, kind="ExternalInput")
    u32_d = nc.dram_tensor("u32", [HF, HC], f16, kind="ExternalInput")
    ident_d = nc.dram_tensor("ident", [128, 128], f32, kind="ExternalInput")
    out_d = nc.dram_tensor("out", [K2, FTW], f32, kind="ExternalOutput")

    with tile.TileContext(nc) as tc, ExitStack() as ctx:
        const_pool = ctx.enter_context(tc.tile_pool(name="const", bufs=1))
        joint_pool = ctx.enter_context(tc.tile_pool(name="joint", bufs=1))
        ft_pool = ctx.enter_context(tc.tile_pool(name="ft", bufs=1))
        res_pool = ctx.enter_context(tc.tile_pool(name="res", bufs=1))
        ps_pool = ctx.enter_context(
            tc.tile_pool(name="ps", bufs=4, space="PSUM"))
        psf_pool = ctx.enter_context(
            tc.tile_pool(name="psf", bufs=1, space="PSUM"))

        # --- lead-in DMAs: tiny mask/iota first so oh can start, then the
        # critical preds quarters; everything else is issued after preds ---
        iota_t = const_pool.tile([128, K * WFB], f16, tag="iota")
        nc.sync.dma_start(iota_t[:], iota_d.ap()[:, :])
        mask_t = []
        for h in range(2):
            mt = const_pool.tile([128, WF], f16, tag=f"mask{h}")
            nc.sync.dma_start(mt[:], mask_d.ap()[h, :, :])
            mask_t.append(mt)

        # --- one-hot of mask (independent of preds; runs in DMA shadow) ---
        # oh[p, k, w] = (mask[p, w] == k), fp16 exact, DVE 2x mode
        oh_t = []
        for h in range(2):
            oh = joint_pool.tile([128, KW], f16, tag=f"oh{h}")
            oh_t.append(oh)
            for b in range(NB):
                ws = b * WFB
                nc.vector.tensor_tensor(
                    oh[:].rearrange("p (k w) -> p k w", k=K)[:, :, ws:ws + WFB],
                    mask_t[h][:, ws:ws + WFB].unsqueeze(1).to_broadcast(
                        [128, K, WFB]),
                    iota_t[:].rearrange("p (k w) -> p k w", k=K),
                    op=mybir.AluOpType.is_equal)

        # --- preds load + argmax-eq + joint, per (half, wf-block) ---
        joint_t = []
        with tc.tile_pool(name="trans", bufs=2) as tr_pool, \
                tc.tile_pool(name="mx", bufs=4) as mx_pool:
            preds_t = []
            for h in range(2):
                hs = h * 128
                pt = tr_pool.tile([128, KW], f32, tag=f"preds{h}")
                preds_t.append(pt)
                for b in range(NB):
                    ws = b * WFB
                    nc.sync.dma_start(
                        pt[:].rearrange("p (k w) -> p k w", k=K)[
                            :, :, ws:ws + WFB],
                        preds_d.ap()[:, hs:hs + 128, ws:ws + WFB].transpose(
                            [1, 0, 2]))

            # feats^T (pix on partitions, hc-major) with ones+pad cols, fp16.
            # Issued after the preds DMAs: only the final matmuls need it.
            ft_big = ft_pool.tile([128, (PIX // 128) * FTW], f16, tag="ftbig")
            nc.sync.dma_start(
                ft_big[:].rearrange("p (x n) -> p x n", x=PIX // 128),
                ft_d.ap().transpose([1, 0, 2]))
            u16_t = []
            u32_t = []
            for h in range(2):
                t16 = const_pool.tile([128, HC], f16, tag=f"u16_{h}")
                nc.sync.dma_start(t16[:], u16_d.ap()[h * 128:(h + 1) * 128, :])
                u16_t.append(t16)
                t32 = const_pool.tile([128, HC], f16, tag=f"u32_{h}")
                nc.sync.dma_start(t32[:], u32_d.ap()[h * 128:(h + 1) * 128, :])
                u32_t.append(t32)
            ident_t = const_pool.tile([128, 128], f32, tag="ident")
            nc.sync.dma_start(ident_t[:], ident_d.ap()[:, :])

            for h in range(2):
                jt = joint_pool.tile([128, KW], f16, tag=f"joint{h}")
                joint_t.append(jt)
                for b in range(NB):
                    ws = b * WFB
                    p3 = preds_t[h][:].rearrange(
                        "p (k w) -> p k w", k=K)[:, :, ws:ws + WFB]
                    # max over k (single strided reduce, k innermost)
                    mxv = mx_pool.tile([128, WFB], f32, tag="mxv")
                    nc.vector.tensor_reduce(
                        mxv[:], p3.transpose([0, 2, 1]),
                        axis=mybir.AxisListType.X, op=mybir.AluOpType.max)
                    # eq = (preds == maxv): fp32 compare, fp16 0/1 out
                    jv = jt[:].rearrange("p (k w) -> p k w", k=K)[
                        :, :, ws:ws + WFB]
                    nc.vector.tensor_tensor(
                        jv, p3,
                        mxv[:].unsqueeze(1).to_broadcast([128, K, WFB]),
                        op=mybir.AluOpType.is_equal)
                    # joint = eq * oh (in place, fp16 2x)
                    ov = oh_t[h][:].rearrange("p (k w) -> p k w", k=K)[
                        :, :, ws:ws + WFB]
                    nc.vector.tensor_tensor(
                        jv, jv, ov, op=mybir.AluOpType.mult)

        if stage <= 1:  # debug: dump joint slice for classes 0..K2-1
            dbg = res_pool.tile([128, K2], f32, tag="dbg")
            nc.scalar.copy(dbg[:], joint_t[0][:, 0:K2])
            nc.sync.dma_start(out_d.ap()[:, 0:128].transpose([1, 0]), dbg[:])

        # ----- stage 1: contract hf.  A[hc, (k, wf)] = Uh^T @ joint -----
        with tc.tile_pool(name="stg", bufs=1) as st_pool:
            a_t = st_pool.tile([64, KW], f32, tag="a")
            for fc in range(0, KW if stage >= 2 else 0, 512):
                w = min(512, KW - fc)
                ps = ps_pool.tile([64, 512], f32, tag="ps")
                nc.tensor.matmul(ps[:, :w], u16_t[0][:, :],
                                 joint_t[0][:, fc:fc + w],
                                 start=True, stop=False)
                nc.tensor.matmul(ps[:, :w], u16_t[1][:, :],
                                 joint_t[1][:, fc:fc + w],
                                 start=False, stop=True)
                nc.scalar.copy(a_t[:, fc:fc + w], ps[:, :w])

            # ----- stage 1.5: transpose A per class -> AT[wf, (wh, hc, k2)]
            at_big = st_pool.tile([128, 2 * HK], f16, tag="at")
            if stage >= 3:
                # zero the k=21 pad column so stage 2 reads clean zeros
                nc.vector.memset(
                    at_big[:].rearrange(
                        "p (w h k) -> p w h k", w=2, h=HC)[:, :, :, K], 0.0)
            for k in range(K if stage >= 3 else 0):
                ps = ps_pool.tile([128, 128], f32, tag="ps")
                for wh in range(2):
                    nc.tensor.transpose(
                        ps[:, wh * 64:(wh + 1) * 64],
                        a_t[:, k * WF + wh * 128: k * WF + wh * 128 + 128],
                        ident_t[:64, :64])
                dst = at_big[:].rearrange(
                    "p (w h k) -> p w h k", w=2, h=HC)[:, :, :, k]
                nc.scalar.copy(dst, ps[:].rearrange("p (w h) -> p w h", w=2))

            # ----- stage 2: contract wf.  B[wc, (hc, k2)] = Uw^T @ AT -----
            b2 = st_pool.tile([128, HK], f16, tag="b2")
            for fc in range(0, HK if stage >= 3 else 0, 512):
                w = min(512, HK - fc)
                ps = ps_pool.tile([64, 512], f32, tag="ps")
                nc.tensor.matmul(ps[:, :w], u32_t[0][:, :],
                                 at_big[:, fc:fc + w],
                                 start=True, stop=False)
                nc.tensor.matmul(ps[:, :w], u32_t[1][:, :],
                                 at_big[:, HK + fc:HK + fc + w],
                                 start=False, stop=True)
                nc.scalar.copy(b2[0:64, fc:fc + w], ps[:, :w])
            if stage >= 3:
                # partitions 64-127 hold B shifted by one hc so a 128-pixel
                # chunk (2 hc rows x 64 wc) is one full-partition stat slice
                nc.sync.dma_start(b2[64:128, 0:HK - K2], b2[0:64, K2:HK])

            # ----- final: out[k, c] = sum_ch b2_ch^T @ ft_ch, + counts ----
            ftv = ft_big[:].rearrange("p (x n) -> p x n", x=PIX // 128)
            if stage >= 4:
                psum_o = psf_pool.tile([K2, FTW], f32, tag="fin")
                for ch in range(PIX // 128):
                    nc.tensor.matmul(
                        psum_o[:, :],
                        b2[:, 2 * ch * K2: 2 * ch * K2 + K2],
                        ftv[:, ch, :],
                        start=(ch == 0), stop=(ch == PIX // 128 - 1))
                outc = res_pool.tile([K2, FTW], f32, tag="outc")
                nc.scalar.copy(outc[:], psum_o[:])
                nc.sync.dma_start(out_d.ap()[:, :], outc[:])

    nc.compile()
    return nc


def _get_program():
    if "nc" not in _PROGRAM_CACHE:
        _PROGRAM_CACHE["nc"] = _build_program()
    return _PROGRAM_CACHE["nc"]


def _host_inputs(feats, preds, masks):
    U = _upsample_matrix(HC, HF)
    u16 = U.astype(np.float16)
    ident = np.eye(128, dtype=np.float32)
    iota = np.broadcast_to(
        np.arange(K, dtype=np.float16)[None, :, None], (128, K, WFB)
    ).reshape(128, K * WFB).copy()

    feats = np.asarray(feats, dtype=np.float32)
    preds = np.asarray(preds, dtype=np.float32)
    masks_f = np.asarray(masks).astype(np.float16).reshape(B, 2, 128, WF)

    # feats^T [pix, c] fp16 with ones + zero-pad cols -> [32, 128, 258]
    ftp = np.empty((B, PIX, FTW), dtype=np.float16)
    ftp[:, :, :C] = feats.reshape(B, C, PIX).transpose(0, 2, 1)
    ftp[:, :, C] = 1.0
    ftp[:, :, C + 1] = 0.0

    in_maps = []
    for b in range(B):
        in_maps.append({
            "preds": np.ascontiguousarray(preds[b]),
            "mask": np.ascontiguousarray(masks_f[b]),
            "iota": iota,
            "ft": np.ascontiguousarray(ftp[b].reshape(PIX // 128, 128, FTW)),
            "u16": u16,
            "u32": u16,
            "ident": ident,
        })
    return in_maps


def kernel(feats, preds, masks, _results_hook=None):
    from concourse.bass_utils import run_bass_kernel_spmd

    nc = _get_program()
    in_maps = _host_inputs(feats, preds, masks)
    res = run_bass_kernel_spmd(nc, in_maps, list(range(N_CORES)))
    if _results_hook is not None:
        _results_hook(res)

    protos = []
    for b in range(B):
        out = res.results[b]["out"]   # [K2, FTW] f32
        sums = out[:K, :C]            # [K, C]
        counts = out[:K, C]           # [K]
        protos.append(sums / (counts + EPS)[:, None])  # [K, C]
    return np.mean(np.stack(protos), axis=0).astype(np.float32)
